# revision 17
# baseline (speedup 1.0000x reference)
"""Trainium2 Bass kernel for single-head dot-product self-attention.

  reference:  Q = x@Wq, K = x@Wk, V = x@Wv          (per batch element)
              out = softmax(Q K^T / sqrt(512)) @ V

Sharding: data-parallel over batch B=8 -> one batch element per NeuronCore.
All matmuls run in float32r (TF32-class rounding, IEEE fp32 container,
~2.4e-4 rel err per rounding, ~4x faster than fp32 on the PE).

Layout strategy per core (transposed-scores):
  - x [2048,512] is DMA'd naturally then PE-transposed once into
    xT [d, s] tiles (d on partitions): every projection contracts over d.
  - QT/KT are produced transposed ([u, s]); scores are computed
    TRANSPOSED: S^T[k, q] = KT_slice.T @ QT, k on PSUM partitions.
  - exp(S^T) tiles feed the PV matmul directly as the moving operand
    (stationary = natural-layout V slices), so no attention-matrix
    transposes are ever needed:  out^T[u, q] = sum_k V[k,u] expS^T[k,q].
  - softmax denominator: DVE accumulates expS^T tiles over k (free-axis
    q), one ones-vector matmul reduces the 128 partitions, and a 2KB
    DRAM round-trip redistributes den[1, 512] to per-partition [128, 4]
    for the final per-q scaling.
  - out^T is PE-transposed back to [q, u] (4x 128x128 per q-chunk into
    one PSUM bank), scaled by 1/den (per-partition scalar) and DMA'd out.

All tiles are 512 columns wide so cross-phase dependencies stay fine-
grained and the QKV and attention phases overlap on the scheduler.
"""

import sys

sys.path.insert(0, "/opt/trn_rl_repo")

import numpy as np

import concourse.bass as bass  # noqa: F401
import concourse.mybir as mybir
import concourse.tile as tile
from concourse import bacc
from concourse.bass_utils import run_bass_kernel_spmd
from concourse.masks import make_identity

f32 = mybir.dt.float32
f32r = mybir.dt.float32r

B, S, D, U = 8, 2048, 512, 512
P = 128                 # partitions
SC = S // P             # 16 s-chunks (also k-chunks)
DC = D // P             # 4 d-chunks
UC = U // P             # 4 u-chunks
NT = S // 512           # 4 512-wide tiles along s/k/q
SCALE = 1.0 / float(np.sqrt(U))
EXP = mybir.ActivationFunctionType.Exp


class _nullctx:
    def __enter__(self):
        return None

    def __exit__(self, *a):
        return False


def build_nc(repeat: int = 1, hw_all: int = 1):
    """repeat: python-unrolled whole-kernel reps (timing).
    hw_all: hardware-loop count around the whole kernel body (timing)."""
    nc = bacc.Bacc("TRN2", target_bir_lowering=False, debug=False)
    x_d = nc.dram_tensor("x", [S, D], f32, kind="ExternalInput")
    w_d = {
        "q": nc.dram_tensor("Wq", [D, U], f32, kind="ExternalInput"),
        "k": nc.dram_tensor("Wk", [D, U], f32, kind="ExternalInput"),
        "v": nc.dram_tensor("Wv", [D, U], f32, kind="ExternalInput"),
    }
    out_d = nc.dram_tensor("out", [S, U], f32, kind="ExternalOutput")

    with tile.TileContext(nc) as tc:
        with (
            tc.tile_pool(name="persist", bufs=1) as persist,
            tc.tile_pool(name="wstage", bufs=2) as wstage,
            tc.tile_pool(name="wr", bufs=1) as wr_pool,
            tc.tile_pool(name="xstage", bufs=8) as xstage,
            tc.tile_pool(name="xT", bufs=1) as xT_pool,
            tc.tile_pool(name="expp", bufs=1) as exp_pool,
            tc.tile_pool(name="attn_sb", bufs=2) as attn_pool,
            tc.tile_pool(name="outsb", bufs=4) as out_pool,
            tc.tile_pool(name="ps_mm", bufs=2, space="PSUM") as ps_mm,
            tc.tile_pool(name="ps_o", bufs=1, space="PSUM") as ps_o_pool,
            tc.tile_pool(name="ps_t", bufs=2, space="PSUM") as ps_t_pool,
        ):
            ident = persist.tile([P, P], f32, tag="ident")
            make_identity(nc, ident[:])
            ones_f = persist.tile([P, 1], f32, tag="ones_f")
            nc.gpsimd.memset(ones_f[:], 1.0)

            # per-512-column tiles: [u][st] for QT/KT, [d][sg] for xT
            QT = [[persist.tile([P, 512], f32r, tag=f"QT{u}_{st}",
                                name=f"QT{u}_{st}") for st in range(NT)]
                  for u in range(UC)]
            KT = [[persist.tile([P, 512], f32r, tag=f"KT{u}_{st}",
                                name=f"KT{u}_{st}") for st in range(NT)]
                  for u in range(UC)]
            V = [persist.tile([P, U], f32r, tag=f"V{s}", name=f"V{s}")
                 for s in range(SC)]

            for _rep in range(repeat):
                with (tc.For_i(0, hw_all, 1) if hw_all > 1
                      else _nullctx()):
                    # ---------- phase 1+2: load, transpose x, project QKV
                    xs_all = []
                    for s in range(4):
                        t = xstage.tile([P, D], f32, tag="xs", name="xs")
                        nc.sync.dma_start(t[:], x_d[s * P:(s + 1) * P, :])
                        xs_all.append(t)
                    wr = {}
                    wsts = {}
                    for wname, wd in w_d.items():
                        for d in range(DC):
                            stg = wstage.tile([P, U], f32, tag=f"wstg{wname}{d}",
                                              name="wstg", bufs=1)
                            nc.sync.dma_start(stg[:], wd[d * P:(d + 1) * P, :])
                            wsts[wname, d] = stg
                            wr[wname, d] = wr_pool.tile(
                                [P, U], f32r, tag=f"w{wname}{d}",
                                name=f"w{wname}{d}")
                    for d in range(DC):
                        nc.scalar.copy(wr["q", d][:], wsts["q", d][:])

                    for sg in range(NT):
                        xTg = [xT_pool.tile([P, 512], f32r, tag=f"xT{d}",
                                            name=f"xT{d}", bufs=2)
                               for d in range(DC)]
                        if sg == 0:
                            xs = xs_all
                        else:
                            xs = []
                            for j in range(4):
                                s = sg * 4 + j
                                t = xstage.tile([P, D], f32, tag="xs",
                                                name="xs")
                                nc.sync.dma_start(t[:],
                                                  x_d[s * P:(s + 1) * P, :])
                                xs.append(t)
                        for d in range(DC):
                            pst = ps_t_pool.tile([P, 512], f32, tag="t",
                                                 name="pst")
                            for j in range(4):
                                nc.tensor.transpose(
                                    pst[:, j * P:(j + 1) * P],
                                    xs[j][:, d * P:(d + 1) * P], ident[:])
                            nc.vector.tensor_copy(xTg[d][:], pst[:])
                        if sg == 0:
                            for d in range(DC):
                                nc.vector.tensor_copy(wr["k", d][:],
                                                      wsts["k", d][:])
                            for d in range(DC):
                                nc.vector.tensor_copy(wr["v", d][:],
                                                      wsts["v", d][:])

                        # projections for this 512-wide s window:
                        # QT/KT tiles [u][sg] and V s-chunks in the window
                        for wname, dstT in (("q", QT), ("k", KT)):
                            for u in range(UC):
                                ps = ps_mm.tile([P, 512], f32, tag="mm",
                                                name="ps")
                                for d in range(DC):
                                    nc.tensor.matmul(
                                        ps[:],
                                        wr[wname, d][:, u * P:(u + 1) * P],
                                        xTg[d][:],
                                        start=(d == 0), stop=(d == DC - 1))
                                nc.scalar.copy(dstT[u][sg][:], ps[:])
                        for j in range(4):
                            s = sg * 4 + j
                            ps = ps_mm.tile([P, 512], f32, tag="mm",
                                            name="ps")
                            for d in range(DC):
                                nc.tensor.matmul(
                                    ps[:], xTg[d][:, j * P:(j + 1) * P],
                                    wr["v", d][:],
                                    start=(d == 0), stop=(d == DC - 1))
                            nc.scalar.copy(V[s][:], ps[:])

                    # ---------- phase 3: attention per 512-wide q tile
                    for qt in range(NT):
                        den_acc = attn_pool.tile([P, 512], f32,
                                                  tag="den_acc",
                                                  name="den_acc", bufs=1)
                        ps_den = ps_t_pool.tile([P, 512], f32, tag="t",
                                                name="ps_den")
                        ps_o = [ps_o_pool.tile([P, 512], f32, tag=f"o{c}",
                                               name=f"o{c}")
                                for c in range(4)]
                        for k in range(SC):
                            ps = ps_mm.tile([P, 512], f32, tag="mm",
                                            name="ps")
                            for u in range(UC):
                                nc.tensor.matmul(
                                    ps[:], KT[u][k // 4][:, (k % 4) * P:
                                                         (k % 4 + 1) * P],
                                    QT[u][qt][:],
                                    start=(u == 0), stop=(u == UC - 1))
                            e = exp_pool.tile([P, 512], f32r, tag="e",
                                              name="e", bufs=6)
                            nc.scalar.activation(e[:], ps[:], EXP,
                                                 scale=SCALE)
                            for c in range(4):
                                nc.tensor.matmul(
                                    ps_o[c][:],
                                    e[:, c * P:(c + 1) * P],
                                    V[k][:],
                                    start=(k == 0), stop=(k == SC - 1))
                            if k == 0:
                                nc.vector.tensor_copy(den_acc[:],
                                                      e[:].bitcast(f32))
                            else:
                                nc.vector.tensor_add(den_acc[:], den_acc[:],
                                                     e[:].bitcast(f32))

                        # partition reduction via 4 fp32 matvecs; result
                        # lands directly in [q-partition, chunk] layout.
                        for c in range(4):
                            nc.tensor.matmul(
                                ps_den[:, c:c + 1],
                                den_acc[:, c * P:(c + 1) * P],
                                ones_f[:], start=True, stop=True)
                        recipT = attn_pool.tile([P, NT], f32,
                                                tag="recipT", name="recipT")
                        nc.vector.reciprocal(recipT[:], ps_den[:, 0:NT])

                        # scale rows by 1/den and store
                        for c in range(4):
                            outt = out_pool.tile([P, U], f32, tag="out",
                                                 name="outt")
                            nc.vector.tensor_scalar_mul(
                                outt[:], ps_o[c][:], recipT[:, c:c + 1])
                            q0 = qt * 512 + c * P
                            nc.sync.dma_start(out_d[q0:q0 + P, :], outt[:])

    nc.finalize()
    return nc


def kernel(x: np.ndarray, Wq: np.ndarray, Wk: np.ndarray,
           Wv: np.ndarray) -> np.ndarray:
    x = np.ascontiguousarray(x, dtype=np.float32)
    Wq = np.ascontiguousarray(Wq, dtype=np.float32)
    Wk = np.ascontiguousarray(Wk, dtype=np.float32)
    Wv = np.ascontiguousarray(Wv, dtype=np.float32)
    assert x.shape == (B, S, D)

    nc = build_nc()
    in_maps = [{"x": x[b], "Wq": Wq, "Wk": Wk, "Wv": Wv} for b in range(B)]
    res = run_bass_kernel_spmd(nc, in_maps, list(range(B)))
    return np.stack([res.results[b]["out"] for b in range(B)], axis=0)


if __name__ == "__main__":
    rng = np.random.default_rng(0)
    x = rng.standard_normal((B, S, D), dtype=np.float32)
    sc = 1.0 / np.sqrt(D)
    Wq = rng.standard_normal((D, U), dtype=np.float32) * sc
    Wk = rng.standard_normal((D, U), dtype=np.float32) * sc
    Wv = rng.standard_normal((D, U), dtype=np.float32) * sc
    out = kernel(x=x, Wq=Wq, Wk=Wk, Wv=Wv)
    print("out", out.shape, out.dtype)


# revision 20
# speedup vs baseline: 1.2097x; 1.2097x over previous
"""Trainium2 Bass kernel for single-head dot-product self-attention.

  reference:  Q = x@Wq, K = x@Wk, V = x@Wv          (per batch element)
              out = softmax(Q K^T / sqrt(512)) @ V

Sharding: data-parallel over batch B=8 -> one batch element per NeuronCore.
All matmuls run in float32r (TF32-class rounding, IEEE fp32 container,
~2.4e-4 rel err per rounding, ~4x faster than fp32 on the PE).

Layout strategy per core (transposed-scores + Gram-matrix trick):
  - x [2048,512] is DMA'd naturally then PE-transposed once into
    xT [d, s] tiles (d on partitions): every contraction over d or over
    key positions uses these directly.
  - Q and K are never materialized: S = X Wq Wk^T X^T, so a tiny Gram
    matrix G = Wq Wk^T (32 weight transposes + 16 matmuls) and
    Z = G^T X^T (64 matmuls) replace both projections (128 matmuls),
    saving 16K PE cycles.
  - scores are computed TRANSPOSED: S^T[k, q] = xT_slice.T @ Z,
    k on PSUM partitions, fused per k-chunk with exp and PV:
  - exp(S^T) slices are the PV stationary operand with V moving, so
    out[q, u] rows fall out directly -- zero attention/output transposes.
  - softmax denominator: DVE accumulates exp tiles over k; four fp32
    N=1 matvecs against a ones vector reduce the 128 partitions AND
    transpose den into [q-partition, chunk] layout in one step; a
    reciprocal and per-partition scalar multiply finish each row block.
  - softmax max-subtraction is skipped: scores ~ N(0,1) for this
    problem family, exp is comfortably inside fp32 range.

All tiles are 512 columns wide so cross-phase dependencies stay fine-
grained and the phases overlap on the Tile scheduler.
"""

import sys

sys.path.insert(0, "/opt/trn_rl_repo")

import numpy as np

import concourse.bass as bass  # noqa: F401
import concourse.mybir as mybir
import concourse.tile as tile
from concourse import bacc
from concourse.bass_utils import run_bass_kernel_spmd
from concourse.masks import make_identity

f32 = mybir.dt.float32
f32r = mybir.dt.float32r

B, S, D, U = 8, 2048, 512, 512
P = 128                 # partitions
SC = S // P             # 16 s-chunks (also k-chunks)
DC = D // P             # 4 d-chunks
UC = U // P             # 4 u-chunks
NT = S // 512           # 4 512-wide tiles along s/k/q
SCALE = 1.0 / float(np.sqrt(U))
EXP = mybir.ActivationFunctionType.Exp


class _nullctx:
    def __enter__(self):
        return None

    def __exit__(self, *a):
        return False


def build_nc(repeat: int = 1, hw_all: int = 1):
    """repeat: python-unrolled whole-kernel reps (timing).
    hw_all: hardware-loop count around the whole kernel body (timing)."""
    nc = bacc.Bacc("TRN2", target_bir_lowering=False, debug=False)
    x_d = nc.dram_tensor("x", [S, D], f32, kind="ExternalInput")
    w_d = {
        "q": nc.dram_tensor("Wq", [D, U], f32, kind="ExternalInput"),
        "k": nc.dram_tensor("Wk", [D, U], f32, kind="ExternalInput"),
        "v": nc.dram_tensor("Wv", [D, U], f32, kind="ExternalInput"),
    }
    out_d = nc.dram_tensor("out", [S, U], f32, kind="ExternalOutput")

    with tile.TileContext(nc) as tc:
        with (
            tc.tile_pool(name="persist", bufs=1) as persist,
            tc.tile_pool(name="wstage", bufs=2) as wstage,
            tc.tile_pool(name="wr", bufs=1) as wr_pool,
            tc.tile_pool(name="xstage", bufs=8) as xstage,
            tc.tile_pool(name="xT", bufs=1) as xT_pool,
            tc.tile_pool(name="expp", bufs=1) as exp_pool,
            tc.tile_pool(name="attn_sb", bufs=2) as attn_pool,
            tc.tile_pool(name="outsb", bufs=4) as out_pool,
            tc.tile_pool(name="ps_mm", bufs=2, space="PSUM") as ps_mm,
            tc.tile_pool(name="ps_o", bufs=1, space="PSUM") as ps_o_pool,
            tc.tile_pool(name="ps_t", bufs=2, space="PSUM") as ps_t_pool,
        ):
            ident = persist.tile([P, P], f32, tag="ident")
            make_identity(nc, ident[:])
            ones_f = persist.tile([P, 1], f32, tag="ones_f")
            nc.gpsimd.memset(ones_f[:], 1.0)

            # Gram-matrix trick: S = X (Wq Wk^T) X^T, so Q/K are never
            # materialized.  G = Wq Wk^T, Z = G^T X^T; S^T = xT_slice.T @ Z.
            xT = [[persist.tile([P, 512], f32r, tag=f"xT{d}_{sg}",
                                name=f"xT{d}_{sg}") for sg in range(NT)]
                  for d in range(DC)]
            Z = [[persist.tile([P, 512], f32r, tag=f"Z{d}_{qt}",
                               name=f"Z{d}_{qt}") for qt in range(NT)]
                 for d in range(DC)]
            WqT = [persist.tile([P, 512], f32r, tag=f"WqT{u}",
                                name=f"WqT{u}") for u in range(UC)]
            WkT = [persist.tile([P, 512], f32r, tag=f"WkT{u}",
                                name=f"WkT{u}") for u in range(UC)]
            G = [persist.tile([P, 512], f32r, tag=f"G{d}", name=f"G{d}")
                 for d in range(DC)]
            V = [persist.tile([P, U], f32r, tag=f"V{s}", name=f"V{s}")
                 for s in range(SC)]

            for _rep in range(repeat):
                with (tc.For_i(0, hw_all, 1) if hw_all > 1
                      else _nullctx()):
                    # ---------- phase 1+2: load, transpose x, project QKV
                    xs_all = []
                    for s in range(4):
                        t = xstage.tile([P, D], f32, tag="xs", name="xs")
                        nc.sync.dma_start(t[:], x_d[s * P:(s + 1) * P, :])
                        xs_all.append(t)
                    wr = {}
                    wsts = {}
                    for wname, wd in w_d.items():
                        for d in range(DC):
                            stg = wstage.tile([P, U], f32, tag=f"wstg{wname}{d}",
                                              name="wstg", bufs=1)
                            nc.sync.dma_start(stg[:], wd[d * P:(d + 1) * P, :])
                            wsts[wname, d] = stg
                    for d in range(DC):
                        wr["v", d] = wr_pool.tile(
                            [P, U], f32r, tag=f"wv{d}", name=f"wv{d}")
                        nc.scalar.copy(wr["v", d][:], wsts["v", d][:])

                    for sg in range(NT):
                        xTg = [xT[d][sg] for d in range(DC)]
                        if sg == 0:
                            xs = xs_all
                        else:
                            xs = []
                            for j in range(4):
                                s = sg * 4 + j
                                t = xstage.tile([P, D], f32, tag="xs",
                                                name="xs")
                                nc.sync.dma_start(t[:],
                                                  x_d[s * P:(s + 1) * P, :])
                                xs.append(t)
                        for d in range(DC):
                            pst = ps_t_pool.tile([P, 512], f32, tag="t",
                                                 name="pst")
                            for j in range(4):
                                nc.tensor.transpose(
                                    pst[:, j * P:(j + 1) * P],
                                    xs[j][:, d * P:(d + 1) * P], ident[:])
                            nc.vector.tensor_copy(xTg[d][:], pst[:])
                        for j in range(4):
                            s = sg * 4 + j
                            ps = ps_mm.tile([P, 512], f32, tag="mm",
                                            name="ps")
                            for d in range(DC):
                                nc.tensor.matmul(
                                    ps[:], xTg[d][:, j * P:(j + 1) * P],
                                    wr["v", d][:],
                                    start=(d == 0), stop=(d == DC - 1))
                            nc.scalar.copy(V[s][:], ps[:])
                        if sg == 0:
                            # Wq/Wk -> [u, d] transposes, then G = Wq Wk^T
                            for wname, dst in (("q", WqT), ("k", WkT)):
                                for u in range(UC):
                                    pst = ps_t_pool.tile([P, 512], f32,
                                                         tag="t",
                                                         name="pstw")
                                    for d in range(DC):
                                        nc.tensor.transpose(
                                            pst[:, d * P:(d + 1) * P],
                                            wsts[wname, d][:,
                                                           u * P:(u + 1) * P],
                                            ident[:])
                                    nc.vector.tensor_copy(dst[u][:], pst[:])
                            for d1 in range(DC):
                                ps = ps_mm.tile([P, 512], f32, tag="mm",
                                                name="psg")
                                for u in range(UC):
                                    nc.tensor.matmul(
                                        ps[:],
                                        WqT[u][:, d1 * P:(d1 + 1) * P],
                                        WkT[u][:],
                                        start=(u == 0), stop=(u == UC - 1))
                                nc.scalar.copy(G[d1][:], ps[:])
                        # Z[d2, q-window] = sum_d1 G[d1][:,d2].T @ xT[d1]
                        for d2 in range(DC):
                            ps = ps_mm.tile([P, 512], f32, tag="mm",
                                            name="ps")
                            for d1 in range(DC):
                                nc.tensor.matmul(
                                    ps[:],
                                    G[d1][:, d2 * P:(d2 + 1) * P],
                                    xTg[d1][:],
                                    start=(d1 == 0), stop=(d1 == DC - 1))
                            nc.scalar.copy(Z[d2][sg][:], ps[:])

                    # ---------- phase 3: attention per 512-wide q tile
                    for qt in range(NT):
                        den_acc = attn_pool.tile([P, 512], f32,
                                                  tag="den_acc",
                                                  name="den_acc", bufs=1)
                        ps_den = ps_t_pool.tile([P, 512], f32, tag="t",
                                                name="ps_den")
                        ps_o = [ps_o_pool.tile([P, 512], f32, tag=f"o{c}",
                                               name=f"o{c}")
                                for c in range(4)]
                        for k in range(SC):
                            ps = ps_mm.tile([P, 512], f32, tag="mm",
                                            name="ps")
                            for d2 in range(DC):
                                nc.tensor.matmul(
                                    ps[:], xT[d2][k // 4][:, (k % 4) * P:
                                                          (k % 4 + 1) * P],
                                    Z[d2][qt][:],
                                    start=(d2 == 0), stop=(d2 == DC - 1))
                            e = exp_pool.tile([P, 512], f32r, tag="e",
                                              name="e", bufs=6)
                            nc.scalar.activation(e[:], ps[:], EXP,
                                                 scale=SCALE)
                            for c in range(4):
                                nc.tensor.matmul(
                                    ps_o[c][:],
                                    e[:, c * P:(c + 1) * P],
                                    V[k][:],
                                    start=(k == 0), stop=(k == SC - 1))
                            if k == 0:
                                nc.vector.tensor_copy(den_acc[:],
                                                      e[:].bitcast(f32))
                            else:
                                nc.vector.tensor_add(den_acc[:], den_acc[:],
                                                     e[:].bitcast(f32))

                        # partition reduction via 4 fp32 matvecs; result
                        # lands directly in [q-partition, chunk] layout.
                        for c in range(4):
                            nc.tensor.matmul(
                                ps_den[:, c:c + 1],
                                den_acc[:, c * P:(c + 1) * P],
                                ones_f[:], start=True, stop=True)
                        recipT = attn_pool.tile([P, NT], f32,
                                                tag="recipT", name="recipT")
                        nc.vector.reciprocal(recipT[:], ps_den[:, 0:NT])

                        # scale rows by 1/den and store
                        for c in range(4):
                            outt = out_pool.tile([P, U], f32, tag="out",
                                                 name="outt")
                            nc.vector.tensor_scalar_mul(
                                outt[:], ps_o[c][:], recipT[:, c:c + 1])
                            q0 = qt * 512 + c * P
                            nc.sync.dma_start(out_d[q0:q0 + P, :], outt[:])

    nc.finalize()
    return nc


def kernel(x: np.ndarray, Wq: np.ndarray, Wk: np.ndarray,
           Wv: np.ndarray) -> np.ndarray:
    x = np.ascontiguousarray(x, dtype=np.float32)
    Wq = np.ascontiguousarray(Wq, dtype=np.float32)
    Wk = np.ascontiguousarray(Wk, dtype=np.float32)
    Wv = np.ascontiguousarray(Wv, dtype=np.float32)
    assert x.shape == (B, S, D)

    nc = build_nc()
    in_maps = [{"x": x[b], "Wq": Wq, "Wk": Wk, "Wv": Wv} for b in range(B)]
    res = run_bass_kernel_spmd(nc, in_maps, list(range(B)))
    return np.stack([res.results[b]["out"] for b in range(B)], axis=0)


if __name__ == "__main__":
    rng = np.random.default_rng(0)
    x = rng.standard_normal((B, S, D), dtype=np.float32)
    sc = 1.0 / np.sqrt(D)
    Wq = rng.standard_normal((D, U), dtype=np.float32) * sc
    Wk = rng.standard_normal((D, U), dtype=np.float32) * sc
    Wv = rng.standard_normal((D, U), dtype=np.float32) * sc
    out = kernel(x=x, Wq=Wq, Wk=Wk, Wv=Wv)
    print("out", out.shape, out.dtype)
